# revision 1
# baseline (speedup 1.0000x reference)
"""Trainium2 Bass kernel: sharded ChempropEncoder on 8 NeuronCores."""


import numpy as np
import scipy.sparse as sp
from scipy.sparse.csgraph import breadth_first_order, connected_components

NCORES = 8
DV, DE, DH, EMB = 72, 14, 300, 256
DEPTH = 3
WIN_CAP = 246      # max edge rows (incl recv) per 128-atom window
SEG_TPW = 2        # segsum tiles per window (128 rows each)
PAD_OFF = 255.0    # one-hot offset value meaning "no atom"


def _round_up(x, m):
    return (x + m - 1) // m * m


def prep(inputs, ncores=NCORES):
    V = np.asarray(inputs["V"], np.float32)
    E = np.asarray(inputs["E"], np.float32)
    src = np.asarray(inputs["edge_src"], np.int64)
    dst = np.asarray(inputs["edge_dst"], np.int64)
    rev = np.asarray(inputs["rev_edge_index"], np.int64)
    batch = np.asarray(inputs["batch"], np.int64)
    n_mols = int(inputs["n_mols"])
    n_atoms = V.shape[0]
    n_bonds = E.shape[0]
    half = n_bonds // 2
    assert np.array_equal(rev, np.arange(n_bonds) ^ 1), "rev must be pair-swap"
    s = src[0::2].copy()
    d = dst[0::2].copy()
    assert np.array_equal(src[1::2], d) and np.array_equal(dst[1::2], s)

    # ---- graph partition: giant-component DFS order + small comps ----
    A = sp.coo_matrix(
        (np.ones(half, np.int8), (s, d)), shape=(n_atoms, n_atoms)
    ).tocsr()
    A = A + A.T
    ncomp, labels = connected_components(A, directed=False)
    sizes = np.bincount(labels, minlength=ncomp)
    giant = int(np.argmax(sizes))
    giant_nodes = np.where(labels == giant)[0]

    # node weight for balancing = pairs owned (as source) + small atom term
    pairs_as_s = np.bincount(s, minlength=n_atoms).astype(np.float64)
    wnode = pairs_as_s + 0.25

    # --- carve the giant component's BFS tree into ~3*ncores connected
    # pieces of roughly equal weight, so tree-edge cut stays tiny ---
    bfs_nodes, preds = breadth_first_order(
        A, int(giant_nodes[0]), directed=False, return_predecessors=True
    )
    bfs_nodes = bfs_nodes.astype(np.int64)
    gsize = len(bfs_nodes)
    wsub = wnode.copy()          # residual subtree weight, updated on carve
    target = wnode[giant_nodes].sum() / (16 * ncores)
    piece_of = np.full(n_atoms, -1, np.int64)
    pieces = []                  # list of node arrays
    # accumulate subtree weights bottom-up (reverse BFS order), carving
    # whenever a subtree's residual weight reaches the target
    carved_root = np.zeros(n_atoms, bool)
    for v in bfs_nodes[::-1]:
        if wsub[v] >= target:
            carved_root[v] = True
        else:
            p = preds[v]
            if p >= 0:
                wsub[p] += wsub[v]
    carved_root[bfs_nodes[0]] = True
    # assign every giant node to its nearest carved ancestor
    root_of = np.full(n_atoms, -1, np.int64)
    for v in bfs_nodes:          # forward BFS order: parents before children
        if carved_root[v]:
            root_of[v] = v
        else:
            root_of[v] = root_of[preds[v]]
    for r in np.unique(root_of[bfs_nodes]):
        members = bfs_nodes[root_of[bfs_nodes] == r]
        pieces.append(members)

    # small components as atomic pieces, in shuffled label order
    rest = np.where(labels != giant)[0]
    rng = np.random.default_rng(12345)
    lab_perm = rng.permutation(ncomp)
    rest_key = lab_perm[labels[rest]]
    rest = rest[np.argsort(rest_key, kind="stable")]
    # group the small-component tail into chunks to limit piece count
    chunk = max(1, len(rest) // (8 * ncores))
    for i in range(0, len(rest), chunk):
        pieces.append(rest[i:i + chunk])

    # greedy pack pieces into cores (largest weight first -> lightest core)
    pw = np.array([wnode[p].sum() for p in pieces])
    order_p = np.argsort(-pw)
    core_weight = np.zeros(ncores)
    core_atoms_list = [[] for _ in range(ncores)]
    for pi in order_p:
        c = int(np.argmin(core_weight))
        core_weight[c] += pw[pi]
        core_atoms_list[c].append(pieces[pi])
    atom_core = np.empty(n_atoms, np.int32)
    core_atoms_raw = []
    for c in range(ncores):
        ac = np.concatenate(core_atoms_list[c])
        core_atoms_raw.append(ac)
        atom_core[ac] = c

    # ---- pair ownership: core of s ----
    pc = atom_core[s]            # owner core per pair
    dc = atom_core[d]            # core of d per pair
    is_cut = pc != dc

    # ---- per-core window packing of atoms (bin-pack by incoming load) ---
    # incoming edge rows targeting atom a on its owner core:
    #   all directed edges with dst == a  (even edges d==a from any owner,
    #   odd edges s==a always local).  Load = in-degree (directed).
    indeg = np.bincount(dst, minlength=n_atoms).astype(np.int64)

    # atoms whose Mv row gets sent to a peer (referenced by another core's
    # cut pair) must land in the FIRST windows so the Mv exchange can start
    # before segsum finishes (static prefix slice).
    ref_atom = np.zeros(n_atoms, bool)
    ref_atom[d[is_cut]] = True

    def _pack(atoms, degs):
        """next-fit decreasing -> (members, loads)"""
        o2 = np.argsort(-degs, kind="stable")
        atoms, degs = atoms[o2], degs[o2]
        nz = degs > 0
        members, loads, cnts = [[]], [0], [0]
        for a, dg in zip(atoms[nz], degs[nz]):
            if loads[-1] + dg > WIN_CAP or cnts[-1] >= 128:
                members.append([])
                loads.append(0)
                cnts.append(0)
            loads[-1] += dg
            cnts[-1] += 1
            members[-1].append(a)
        zi = 0
        zeros = atoms[~nz]
        for wi in range(len(members)):
            take = min(128 - cnts[wi], len(zeros) - zi)
            if take > 0:
                members[wi].extend(zeros[zi:zi + take])
                cnts[wi] += take
                zi += take
        while zi < len(zeros):
            take = min(128, len(zeros) - zi)
            members.append(list(zeros[zi:zi + take]))
            loads.append(0)
            cnts.append(take)
            zi += take
        return members, loads

    core = [dict() for _ in range(ncores)]
    nwin_per_core = []
    nrefwin_per_core = []
    for c in range(ncores):
        ac = core_atoms_raw[c]
        is_ref = ref_atom[ac]
        m1, l1 = _pack(ac[is_ref], indeg[ac[is_ref]])
        m2, l2 = _pack(ac[~is_ref], indeg[ac[~is_ref]])
        win_members = m1 + m2
        win_load = l1 + l2
        nwin_per_core.append(len(win_members))
        nrefwin_per_core.append(len(m1))
        core[c]["win_members"] = win_members
        core[c]["win_load"] = win_load
    REF_WIN = max(nrefwin_per_core)

    NWIN = max(nwin_per_core)
    A_pad = NWIN * 128
    g2l = np.full(n_atoms, -1, np.int64)   # global atom -> local id (win*128+slot)
    for c in range(ncores):
        wm = core[c]["win_members"]
        loc_atoms = np.full(A_pad, -1, np.int64)
        for wi, members in enumerate(wm):
            for si, a in enumerate(members):
                lid = wi * 128 + si
                g2l[a] = lid
                loc_atoms[lid] = a
        core[c]["loc_atoms"] = loc_atoms

    # ---- per-core pair lists ----
    # order: [non-cut pairs | cut pairs sorted by (core(d), g2l[d]) | dummy]
    P_c = np.bincount(pc, minlength=ncores)
    P_pad = _round_up(int(P_c.max()), 128)
    n_pair_tiles = P_pad // 128

    for c in range(ncores):
        mine = np.where(pc == c)[0]
        m_cut = is_cut[mine]
        noncut = mine[~m_cut]
        cut = mine[m_cut]
        ckey = dc[cut] * (A_pad + 1) + g2l[d[cut]]
        cut = cut[np.argsort(ckey, kind="stable")]
        # cut pairs FIRST: their H rows are produced early so the next
        # H-exchange overlaps with the rest of the compute pass
        plist = np.concatenate([cut, noncut])
        core[c]["pairs"] = plist           # global pair ids, len P_c[c]
        core[c]["n_cut"] = len(cut)
        core[c]["cut_pairs"] = cut
    CUT_PAD = _round_up(max(1, max(core[c]["n_cut"] for c in range(ncores))),
                        128)
    # tiles whose pairs are all non-cut on EVERY core (static prefix slice
    # property: pairs < n_cut are cut, so tiles with index >= CUT_PAD/128
    # and tile_end <= min_c(n_noncut + n_cut)... use conservative bound)
    # tile t is "cut-free" iff 128*t >= n_cut for all cores
    cut_tile_hi = CUT_PAD // 128   # tiles below this may contain cut pairs

    # ---- H exchange lists (even-edge H rows of cut pairs -> core(d)) ----
    # sender c1 block for peer c2: its cut pairs with dc==c2, sorted by
    # g2l[d] (receiver local id).  Send gather idx = even row index of the
    # pair in sender's layout (= position in plist).
    send_H = [[None] * ncores for _ in range(ncores)]   # [c1][c2] = pair ids
    for c1 in range(ncores):
        cut = core[c1]["cut_pairs"]
        for c2 in range(ncores):
            sel = cut[dc[cut] == c2]
            sel = sel[np.argsort(g2l[d[sel]], kind="stable")]
            send_H[c1][c2] = sel
    B_H = _round_up(max(1, max(len(send_H[a][b]) for a in range(ncores)
                               for b in range(ncores))), 128)

    # ---- Mv exchange lists (Mv[d] rows from core(d) -> pair owner) ----
    send_M = [[None] * ncores for _ in range(ncores)]   # [c2][c1] = atom ids
    for c1 in range(ncores):
        cut = core[c1]["cut_pairs"]
        for c2 in range(ncores):
            atoms = np.unique(d[cut[dc[cut] == c2]])
            send_M[c2][c1] = atoms
    B_M = _round_up(max(1, max(len(send_M[a][b]) for a in range(ncores)
                               for b in range(ncores))), 128)

    H_rows = 2 * P_pad + ncores * B_H     # H_big row count
    XM_rows = A_pad + ncores * B_M

    n_hs_tiles = ncores * B_H // 128
    n_ms_tiles = ncores * B_M // 128
    n_seg_tiles = NWIN * SEG_TPW

    # ---- per-core static arrays ----
    meta = dict(
        ncores=ncores, P_pad=P_pad, A_pad=A_pad, NWIN=NWIN, B_H=B_H,
        B_M=B_M, H_rows=H_rows, XM_rows=XM_rows, n_pair_tiles=n_pair_tiles,
        n_seg_tiles=n_seg_tiles, n_hs_tiles=n_hs_tiles,
        n_ms_tiles=n_ms_tiles, n_mols=n_mols,
        CUT_PAD=CUT_PAD, cut_tile_hi=cut_tile_hi, MS_PAD=REF_WIN * 128,
    )

    for c in range(ncores):
        plist = core[c]["pairs"]
        np_c = len(plist)
        pad = P_pad - np_c
        s_c = s[plist]
        d_c = d[plist]

        # features, transposed: [86, 2*P_pad]; even rows then odd rows
        VE = np.zeros((2 * P_pad, DV + DE), np.float32)
        VE[:np_c, :DV] = V[s_c]                 # even edges (s->d): V[src]=V[s]
        VE[:np_c, DV:] = E[plist * 2]
        VE[P_pad:P_pad + np_c, :DV] = V[d_c]    # odd edges (d->s): V[d]
        VE[P_pad:P_pad + np_c, DV:] = E[plist * 2 + 1]
        core[c]["VE_T"] = np.ascontiguousarray(VE.T)

        # gather indices for Mv[src] per edge row
        idxA = np.zeros(P_pad, np.int32)        # even edges: Mv[s] always local
        idxA[:np_c] = g2l[s_c]
        idxB = np.zeros(P_pad, np.int32)        # odd edges: Mv[d] local or recv
        loc_mask = dc[plist] == c
        idxB[:np_c][loc_mask] = g2l[d_c[loc_mask]]
        # remote: A_pad + q*B_M + pos in send_M[q][c]
        for q in range(ncores):
            if q == c:
                continue
            atoms_q = send_M[q][c]
            if len(atoms_q) == 0:
                continue
            lookup = {a: i for i, a in enumerate(atoms_q)}
            sel = np.where(~loc_mask & (dc[plist] == q))[0]
            idxB[sel] = A_pad + q * B_M + np.array(
                [lookup[a] for a in d_c[sel]], np.int64)
        core[c]["idxA"] = idxA.reshape(n_pair_tiles, 128)
        core[c]["idxB"] = idxB.reshape(n_pair_tiles, 128)
        core[c]["idxAB"] = np.stack(
            [idxA.reshape(n_pair_tiles, 128),
             idxB.reshape(n_pair_tiles, 128)], axis=2)  # [T,128,2]

        # ---- segsum tiles: per window, gather rows + one-hot offsets ----
        # edge rows targeting local atom a (local id L = win*128+slot):
        #   even rows: positions p in plist with dc==c, row id = p
        #   odd rows:  all positions p, row id = P_pad + p (dst = s, local)
        #   recv rows: 2*P_pad + q*B_H + j for send_H[q][c][j] (dst = d)
        tgt_rows = [[] for _ in range(NWIN)]    # row ids into H_big
        tgt_off = [[] for _ in range(NWIN)]     # slot within window
        lid_d = g2l[d_c]
        for p in np.where(loc_mask)[0]:
            L = lid_d[p]
            tgt_rows[L // 128].append(2 * p)
            tgt_off[L // 128].append(L % 128)
        lid_s = g2l[s_c]
        for p in range(np_c):
            L = lid_s[p]
            tgt_rows[L // 128].append(2 * p + 1)
            tgt_off[L // 128].append(L % 128)
        for q in range(ncores):
            if q == c:
                continue
            pl = send_H[q][c]
            Ls = g2l[d[pl]]
            for j, L in enumerate(Ls):
                tgt_rows[L // 128].append(2 * P_pad + q * B_H + j)
                tgt_off[L // 128].append(L % 128)

        seg_idx = np.zeros((n_seg_tiles, 128), np.int32)
        seg_off = np.full((n_seg_tiles, 128), PAD_OFF, np.float32)
        for wi in range(NWIN):
            rows = tgt_rows[wi]
            offs = tgt_off[wi]
            assert len(rows) <= SEG_TPW * 128, (
                f"window overflow core {c} win {wi}: {len(rows)}")
            for j, (r, o) in enumerate(zip(rows, offs)):
                t = wi * SEG_TPW + j // 128
                seg_idx[t, j % 128] = r
                seg_off[t, j % 128] = o
        core[c]["seg_idx"] = seg_idx
        core[c]["seg_off"] = seg_off

        # ---- H-send gather idx (even rows of cut pairs, grouped by peer) --
        hs_idx = np.zeros(ncores * B_H, np.int32)
        pos_of_pair = {pid: i for i, pid in enumerate(plist)}
        for q in range(ncores):
            pl = send_H[c][q]
            for j, pid in enumerate(pl):
                hs_idx[q * B_H + j] = 2 * pos_of_pair[pid]  # even flat row
        core[c]["hs_idx"] = hs_idx.reshape(n_hs_tiles, 128)

        # ---- Mv-send gather idx (local Mv rows, grouped by peer) ----
        ms_idx = np.zeros(ncores * B_M, np.int32)
        for q in range(ncores):
            atoms_q = send_M[c][q]
            ms_idx[q * B_M:q * B_M + len(atoms_q)] = g2l[atoms_q]
        assert ms_idx.max() < REF_WIN * 128, (c, ms_idx.max(), REF_WIN)
        core[c]["ms_idx"] = ms_idx.reshape(n_ms_tiles, 128)

        # ---- readout: Vown^T ----
        loc_atoms = core[c]["loc_atoms"]
        VO = np.zeros((A_pad, DV), np.float32)
        real = loc_atoms >= 0
        VO[real] = V[loc_atoms[real]]
        core[c]["VownT"] = np.ascontiguousarray(VO.T)

        # ---- molecule pass: windows of 128 mols, gather Hv rows ----
        mol_of_loc = np.full(A_pad, -1, np.int64)
        mol_of_loc[real] = batch[loc_atoms[real]]
        core[c]["mol_of_loc"] = mol_of_loc

    # mol tiles: fixed tiles per 128-mol window (global max)
    NMW = _round_up(n_mols, 128) // 128
    tpw_needed = 1
    mol_lists = []
    for c in range(ncores):
        mol_of_loc = core[c]["mol_of_loc"]
        lists = [[] for _ in range(NMW)]
        for L in np.where(mol_of_loc >= 0)[0]:
            lists[int(mol_of_loc[L]) // 128].append(L)
        mol_lists.append(lists)
        for li in lists:
            tpw_needed = max(tpw_needed, (len(li) + 127) // 128)
    T_MOL = tpw_needed
    n_mol_tiles = NMW * T_MOL
    meta["NMW"] = NMW
    meta["T_MOL"] = T_MOL
    meta["n_mol_tiles"] = n_mol_tiles

    mols_per_core = n_mols // ncores
    counts = np.bincount(batch, minlength=n_mols).astype(np.float32)
    inv_cnt = 1.0 / np.maximum(counts, 1.0)
    meta["mols_per_core"] = mols_per_core

    for c in range(ncores):
        mol_idx = np.zeros((n_mol_tiles, 128), np.int32)
        mol_off = np.full((n_mol_tiles, 128), PAD_OFF, np.float32)
        for wi in range(NMW):
            li = mol_lists[c][wi]
            for j, L in enumerate(li):
                t = wi * T_MOL + j // 128
                mol_idx[t, j % 128] = L
                mol_off[t, j % 128] = int(core[c]["mol_of_loc"][L]) % 128
        core[c]["mol_idx"] = mol_idx
        core[c]["mol_off"] = mol_off
        lo = c * mols_per_core
        ic = inv_cnt[lo:lo + mols_per_core]
        core[c]["inv_cnt"] = np.ascontiguousarray(
            ic.reshape(mols_per_core // 128, 128).T.astype(np.float32))

    return meta, core


# ------------------------------------------------------------------
# numpy simulator of the exact device algorithm (fp32, no bf16)
# ------------------------------------------------------------------
def simulate(meta, core, inputs):
    W_i = np.asarray(inputs["W_i"], np.float32)
    W_h = np.asarray(inputs["W_h"], np.float32)
    W_o = np.asarray(inputs["W_o"], np.float32)
    b_o = np.asarray(inputs["b_o"], np.float32)
    W1 = np.asarray(inputs["W1"], np.float32)
    b1 = np.asarray(inputs["b1"], np.float32)
    W2 = np.asarray(inputs["W2"], np.float32)
    b2 = np.asarray(inputs["b2"], np.float32)
    nc_ = meta["ncores"]
    P_pad, A_pad = meta["P_pad"], meta["A_pad"]
    B_H, B_M = meta["B_H"], meta["B_M"]
    H_rows, XM_rows = meta["H_rows"], meta["XM_rows"]
    NWIN, SEG = meta["NWIN"], meta["n_seg_tiles"]

    H0 = []
    H = []
    for c in range(nc_):
        h0b = core[c]["VE_T"].T @ W_i         # [2P_pad, DH] block layout
        # interleave: flat row 2i = even pair i, 2i+1 = odd pair i
        h0 = np.empty_like(h0b)
        h0[0::2] = h0b[:P_pad]
        h0[1::2] = h0b[P_pad:]
        H0.append(h0)
        H.append(np.maximum(h0, 0.0))

    def exchange_H(H):
        HR = [np.zeros((nc_ * B_H, DH), np.float32) for _ in range(nc_)]
        for c in range(nc_):
            hs = np.zeros((nc_ * B_H, DH), np.float32)
            flat = core[c]["hs_idx"].reshape(-1)
            hs[:] = H[c][flat]                 # gather (pads -> row 0, unused)
            for q in range(nc_):
                HR[q][c * B_H:(c + 1) * B_H] = hs[q * B_H:(q + 1) * B_H]
        return HR

    def segsum(H, HR):
        XM = [np.zeros((XM_rows, DH), np.float32) for _ in range(nc_)]
        for c in range(nc_):
            Hbig = np.concatenate([H[c], HR[c]], axis=0)
            assert Hbig.shape[0] == H_rows
            for t in range(SEG):
                rows = Hbig[core[c]["seg_idx"][t]]        # [128, DH]
                off = core[c]["seg_off"][t]
                onehot = (off[:, None] == np.arange(128)[None, :]).astype(
                    np.float32)                            # [128 rows,128 slot]
                XM[c][(t // 2) * 128:(t // 2) * 128 + 128] += onehot.T @ rows
        return XM

    def exchange_M(XM):
        for c_from in range(nc_):
            ms = XM[c_from][core[c_from]["ms_idx"].reshape(-1)]
            for q in range(nc_):
                XM[q][A_pad + c_from * B_M:A_pad + (c_from + 1) * B_M] = \
                    ms[q * B_M:(q + 1) * B_M]
        return XM

    for it in range(DEPTH - 1):
        HR = exchange_H(H)
        XM = segsum(H, HR)
        XM = exchange_M(XM)
        Hn = []
        for c in range(nc_):
            MvA = XM[c][core[c]["idxA"].reshape(-1)]      # [P_pad, DH]
            MvB = XM[c][core[c]["idxB"].reshape(-1)]
            MA = MvA - H[c][1::2]                         # rev of even = odd
            MB = MvB - H[c][0::2]                         # rev of odd = even
            M = np.empty_like(H[c])
            M[0::2] = MA
            M[1::2] = MB
            Hn.append(np.maximum(H0[c] + M @ W_h, 0.0))
        H = Hn

    HR = exchange_H(H)
    XM = segsum(H, HR)

    Zpart = []
    for c in range(nc_):
        Mv = XM[c][:A_pad]
        X = np.concatenate([core[c]["VownT"].T, Mv], axis=1)
        Hv = np.maximum(X @ W_o + b_o, 0.0)
        # zero out hole atoms (device does this via mol one-hot exclusion)
        Z = np.zeros((meta["NMW"] * 128, DH), np.float32)
        for t in range(meta["n_mol_tiles"]):
            rows = Hv[core[c]["mol_idx"][t]]
            off = core[c]["mol_off"][t]
            onehot = (off[:, None] == np.arange(128)[None, :]).astype(np.float32)
            w = t // meta["T_MOL"]
            Z[w * 128:w * 128 + 128] += onehot.T @ rows
        Zpart.append(Z[:meta["n_mols"]])

    Ztot = np.sum(Zpart, axis=0)                           # [n_mols, DH]
    out = np.zeros((meta["n_mols"], EMB), np.float32)
    mpc = meta["mols_per_core"]
    for c in range(nc_):
        Zc = Ztot[c * mpc:(c + 1) * mpc]
        ic = core[c]["inv_cnt"].T.reshape(-1)[:mpc]
        Zc = Zc * ic[:, None]
        z1 = np.maximum(Zc @ W1 + b1, 0.0)
        out[c * mpc:(c + 1) * mpc] = z1 @ W2 + b2
    return out




import numpy as np
import ml_dtypes

import concourse.bass as bass
import concourse.mybir as mybir
import concourse.tile as tile
from concourse import bacc

BF = mybir.dt.bfloat16
F32 = mybir.dt.float32
I32 = mybir.dt.int32
RELU = mybir.ActivationFunctionType.Relu
EQ = mybir.AluOpType.is_equal

DV, DE, DH, EMB = 72, 14, 300, 256
DVE_ = DV + DE
DEPTH = 3
K_CHUNKS = [(0, 128), (128, 128), (256, 44)]        # DH=300 contraction split
KO_CHUNKS = [(0, 72), (72, 128), (200, 128), (328, 44)]  # DV+DH=372 split
K1_CHUNKS = [(0, 128), (128, 128), (256, 44)]       # DH -> EMB
K2_CHUNKS = [(0, 128), (128, 128)]                  # EMB -> EMB
VB = 4                                               # pair tiles per VE load


def build_kernel(meta, ncores=8):
    P_pad = meta["P_pad"]
    A_pad = meta["A_pad"]
    NWIN = meta["NWIN"]
    B_H, B_M = meta["B_H"], meta["B_M"]
    H_rows, XM_rows = meta["H_rows"], meta["XM_rows"]
    HP = H_rows // 2                  # rows of the [*, 600] paired tensor
    n_pair = meta["n_pair_tiles"]
    n_seg = meta["n_seg_tiles"]
    n_hs, n_ms = meta["n_hs_tiles"], meta["n_ms_tiles"]
    NMW, T_MOL, n_mol = meta["NMW"], meta["T_MOL"], meta["n_mol_tiles"]
    MPC = meta["mols_per_core"]
    NZT = MPC // 128
    CUT_PAD = meta["CUT_PAD"]
    cut_tile_hi = meta["cut_tile_hi"]
    MS_PAD = meta["MS_PAD"]
    RG = [list(range(ncores))]

    nc = bacc.Bacc("TRN2", target_bir_lowering=False, debug=False,
                   num_devices=ncores)

    # ---- I/O ----
    ve_t = nc.dram_tensor("VE_T", [DVE_, 2 * P_pad], BF, kind="ExternalInput")
    vown_t = nc.dram_tensor("VownT", [DV, A_pad], BF, kind="ExternalInput")
    seg_idx = nc.dram_tensor("seg_idx", [128, n_seg], I32, kind="ExternalInput")
    seg_off = nc.dram_tensor("seg_off", [128, n_seg], F32, kind="ExternalInput")
    idx_ab = nc.dram_tensor("idxAB", [128, 2 * n_pair], I32,
                            kind="ExternalInput")
    hs_i = nc.dram_tensor("hs_idx", [128, n_hs], I32, kind="ExternalInput")
    ms_i = nc.dram_tensor("ms_idx", [128, n_ms], I32, kind="ExternalInput")
    mol_i = nc.dram_tensor("mol_idx", [128, n_mol], I32, kind="ExternalInput")
    mol_o = nc.dram_tensor("mol_off", [128, n_mol], F32, kind="ExternalInput")
    inv_c = nc.dram_tensor("inv_cnt", [128, NZT], F32, kind="ExternalInput")
    wi_d = nc.dram_tensor("Wi", [DVE_, DH], BF, kind="ExternalInput")
    wh_d = nc.dram_tensor("Wh", [DH, DH], BF, kind="ExternalInput")
    wo_d = nc.dram_tensor("Wo", [DV + DH, DH], BF, kind="ExternalInput")
    w1_d = nc.dram_tensor("W1", [DH, EMB], BF, kind="ExternalInput")
    w2_d = nc.dram_tensor("W2", [EMB, EMB], BF, kind="ExternalInput")
    bo_d = nc.dram_tensor("bo", [1, DH], F32, kind="ExternalInput")
    b1_d = nc.dram_tensor("b1", [1, EMB], F32, kind="ExternalInput")
    b2_d = nc.dram_tensor("b2", [1, EMB], F32, kind="ExternalInput")
    out_d = nc.dram_tensor("out", [MPC, EMB], F32, kind="ExternalOutput")

    with tile.TileContext(nc) as tc:
        with (
            tc.tile_pool(name="const", bufs=1) as cp,
            tc.tile_pool(name="sb", bufs=4) as sb,
            tc.tile_pool(name="ps", bufs=4, space="PSUM") as pp,
            tc.tile_pool(name="dram", bufs=1, space="DRAM") as dp,
        ):
            # ---------- constants into SBUF ----------
            _cn = [0]

            def cload(dram_t, shape, dtype, sl=None):
                _cn[0] += 1
                t_ = cp.tile(shape, dtype, tag=f"c{_cn[0]}")
                src = dram_t.ap() if sl is None else dram_t.ap()[sl]
                nc.sync.dma_start(out=t_[:], in_=src)
                return t_

            wi_sb = cload(wi_d, [DVE_, DH], BF)
            wh_sb = [cload(wh_d, [kw, DH], BF, np.s_[k0:k0 + kw, :])
                     for k0, kw in K_CHUNKS]
            wo_sb = [cload(wo_d, [kw, DH], BF, np.s_[k0:k0 + kw, :])
                     for k0, kw in KO_CHUNKS]
            w1_sb = [cload(w1_d, [kw, EMB], BF, np.s_[k0:k0 + kw, :])
                     for k0, kw in K1_CHUNKS]
            w2_sb = [cload(w2_d, [kw, EMB], BF, np.s_[k0:k0 + kw, :])
                     for k0, kw in K2_CHUNKS]
            bo_sb = cload(bo_d, [1, DH], F32)
            b1_sb = cload(b1_d, [1, EMB], F32)
            b2_sb = cload(b2_d, [1, EMB], F32)
            segi_sb = cload(seg_idx, [128, n_seg], I32)
            sego_sb = cload(seg_off, [128, n_seg], F32)
            ixab_sb = cload(idx_ab, [128, 2 * n_pair], I32)
            hsi_sb = cload(hs_i, [128, n_hs], I32)
            msi_sb = cload(ms_i, [128, n_ms], I32)
            moli_sb = cload(mol_i, [128, n_mol], I32)
            molo_sb = cload(mol_o, [128, n_mol], F32)
            inv_sb = cload(inv_c, [128, NZT], F32)

            # full [128,128] iota tiles (free ramp / partition ramp)
            iota_i = cp.tile([128, 128], I32)
            nc.gpsimd.iota(iota_i[:], pattern=[[1, 128]], base=0,
                           channel_multiplier=0)
            iota_f = cp.tile([128, 128], F32)
            nc.vector.tensor_copy(out=iota_f[:], in_=iota_i[:])
            iop_i = cp.tile([128, 128], I32)
            nc.gpsimd.iota(iop_i[:], pattern=[[0, 128]], base=0,
                           channel_multiplier=1)
            iop_f = cp.tile([128, 128], F32)
            nc.vector.tensor_copy(out=iop_f[:], in_=iop_i[:])
            ident = cp.tile([128, 128], BF)
            nc.vector.tensor_tensor(
                out=ident[:], in0=iop_f[:], in1=iota_f[:], op=EQ)
            # biases broadcast to all partitions once
            bo_bc = cp.tile([128, DH], F32)
            nc.gpsimd.partition_broadcast(bo_bc[:], bo_sb[:])
            b1_bc = cp.tile([128, EMB], F32)
            nc.gpsimd.partition_broadcast(b1_bc[:], b1_sb[:])
            b2_bc = cp.tile([128, EMB], F32)
            nc.gpsimd.partition_broadcast(b2_bc[:], b2_sb[:])

            # ---------- DRAM intermediates ----------
            h0_d = dp.tile([P_pad, 2 * DH], BF)       # [pair, even|odd]
            h_ping = dp.tile([HP, 2 * DH], BF)
            h_pong = dp.tile([HP, 2 * DH], BF)
            xm_d = dp.tile([XM_rows, DH], BF)
            hs_d = dp.tile([ncores * B_H, DH], BF)
            hr_d = dp.tile([ncores * B_H, DH], BF)
            ms_d = dp.tile([ncores * B_M, DH], BF)
            mr_d = dp.tile([ncores * B_M, DH], BF)
            hv_d = dp.tile([A_pad, DH], BF)
            zp_d = dp.tile([NMW * 128, DH], F32)
            zr_d = dp.tile([MPC, DH], F32)

            def hflat(h, hi=None):
                """[HP,600] tensor viewed as [2*HP,300] flat edge rows."""
                v = h[:].rearrange("a (b c) -> (a b) c", b=2)
                if hi is not None:
                    v = v[0:hi, :]
                return v

            # ---------- helpers ----------
            def onehot(col):
                oh = sb.tile([128, 128], BF, tag="oh")
                nc.vector.tensor_tensor(
                    out=oh[:], in0=sego_sb[:, col:col + 1].to_broadcast(
                        [128, 128]),
                    in1=iota_f[:], op=EQ)
                return oh

            def gather_multi(src_ap, idx_ap, k, tag, dwidth=DH, bufs=3):
                """out[p, j, :] = src[idx[p, j], :]  -> [128, k*dwidth]

                One single-column indirect DMA per j (multi-column offset
                APs miscompute on real HW).
                """
                g = sb.tile([128, k * dwidth], BF, tag=tag, bufs=bufs)
                for j in range(k):
                    nc.gpsimd.indirect_dma_start(
                        out=g[:, j * dwidth:(j + 1) * dwidth],
                        out_offset=None, in_=src_ap,
                        in_offset=bass.IndirectOffsetOnAxis(
                            ap=idx_ap[:, j:j + 1], axis=0))
                return g

            def transpose300(m_ap, tag):
                """[128, 300] -> M^T chunks in one [128, 384] bf16 tile."""
                mt = sb.tile([128, 3 * 128], BF, tag=tag)
                for ci, (k0, kw) in enumerate(K_CHUNKS):
                    tp = pp.tile([128, 128], BF, tag="tp", bufs=3)
                    nc.tensor.transpose(
                        out=tp[0:kw, :], in_=m_ap[:, k0:k0 + kw],
                        identity=ident[:])
                    nc.vector.tensor_copy(out=mt[0:kw, 128 * ci:128 * (ci + 1)],
                                          in_=tp[0:kw, :])
                return mt

            # ---------- phase 0: H0 + H1 = relu(H0) ----------
            def phase0():
                for t0 in range(0, n_pair, VB):
                    nvb = min(VB, n_pair - t0)
                    vee = sb.tile([DVE_, VB * 128], BF, tag="vee")
                    nc.sync.dma_start(
                        out=vee[:, 0:nvb * 128],
                        in_=ve_t.ap()[:, 128 * t0:128 * (t0 + nvb)])
                    veo = sb.tile([DVE_, VB * 128], BF, tag="veo")
                    nc.sync.dma_start(
                        out=veo[:, 0:nvb * 128],
                        in_=ve_t.ap()[:, P_pad + 128 * t0:
                                      P_pad + 128 * (t0 + nvb)])
                    for j in range(nvb):
                        t = t0 + j
                        h0sb = sb.tile([128, 2 * DH], BF, tag="h0w")
                        h1sb = sb.tile([128, 2 * DH], BF, tag="h1w")
                        for half, vsb in ((0, vee), (1, veo)):
                            ph = pp.tile([128, DH], F32, tag="mm")
                            nc.tensor.matmul(
                                ph[:], lhsT=vsb[:, 128 * j:128 * (j + 1)],
                                rhs=wi_sb[:], start=True, stop=True)
                            sl = np.s_[:, half * DH:(half + 1) * DH]
                            nc.vector.tensor_copy(out=h0sb[sl], in_=ph[:])
                            nc.scalar.activation(out=h1sb[sl], in_=ph[:],
                                                 func=RELU)
                        nc.scalar.dma_start(
                            out=h0_d[128 * t:128 * (t + 1), :], in_=h0sb[:])
                        nc.scalar.dma_start(
                            out=h_ping[128 * t:128 * (t + 1), :], in_=h1sb[:])

            # ---------- H row exchange (overlaps with compute tail) ------
            def exchange_h(h_cur):
                g = gather_multi(hflat(h_cur, 2 * CUT_PAD),
                                 hsi_sb[:, 0:n_hs], n_hs, "hsg", bufs=1)
                nc.scalar.dma_start(
                    out=hs_d[:].rearrange("(k p) d -> p k d", p=128),
                    in_=g[:].rearrange("p (k d) -> p k d", k=n_hs))
                nc.gpsimd.collective_compute(
                    "AllToAll", mybir.AluOpType.bypass, replica_groups=RG,
                    ins=[hs_d.opt()], outs=[hr_d.opt()])
                nc.sync.dma_start(
                    out=h_cur[P_pad:P_pad + ncores * B_H // 2, :],
                    in_=hr_d[:].rearrange("(a b) c -> a (b c)", b=2))

            # ---------- segment sum into XM[0:A_pad] ----------
            def segsum(h_cur):
                hfv = hflat(h_cur)
                for w in range(NWIN):
                    g = gather_multi(hfv, segi_sb[:, 2 * w:2 * w + 2], 2,
                                     "segg")
                    ps = pp.tile([128, DH], F32, tag="mm")
                    for j in range(2):
                        oh = onehot(2 * w + j)
                        nc.tensor.matmul(ps[:], lhsT=oh[:],
                                         rhs=g[:, j * DH:(j + 1) * DH],
                                         start=(j == 0), stop=(j == 1))
                    mv = sb.tile([128, DH], BF, tag="mvw")
                    nc.vector.tensor_copy(out=mv[:], in_=ps[:])
                    nc.scalar.dma_start(out=xm_d[128 * w:128 * (w + 1), :],
                                        in_=mv[:])

            # ---------- Mv row exchange (overlaps with segsum tail) ------
            def exchange_m():
                g = gather_multi(xm_d[0:MS_PAD, :], msi_sb[:, 0:n_ms], n_ms,
                                 "msg", bufs=1)
                nc.scalar.dma_start(
                    out=ms_d[:].rearrange("(k p) d -> p k d", p=128),
                    in_=g[:].rearrange("p (k d) -> p k d", k=n_ms))
                nc.gpsimd.collective_compute(
                    "AllToAll", mybir.AluOpType.bypass, replica_groups=RG,
                    ins=[ms_d.opt()], outs=[mr_d.opt()])
                nc.sync.dma_start(out=xm_d[A_pad:A_pad + ncores * B_M, :],
                                  in_=mr_d[:])

            # ---------- message-passing update ----------
            def compute(h_cur, h_nxt):
                for t in range(n_pair):
                    src_ap = (xm_d[:] if t < cut_tile_hi
                              else xm_d[0:A_pad, :])
                    mvab = gather_multi(src_ap,
                                        ixab_sb[:, 2 * t:2 * t + 2], 2, "mvab")
                    h2 = sb.tile([128, 2 * DH], BF, tag="h2")
                    nc.sync.dma_start(
                        out=h2[:], in_=h_cur[128 * t:128 * (t + 1), :])
                    h02 = sb.tile([128, 2 * DH], BF, tag="h02")
                    nc.sync.dma_start(
                        out=h02[:], in_=h0_d[128 * t:128 * (t + 1), :])
                    hn = sb.tile([128, 2 * DH], BF, tag="hn")
                    for half in (0, 1):
                        m = sb.tile([128, DH], BF, tag="m")
                        nc.vector.tensor_sub(
                            out=m[:],
                            in0=mvab[:, half * DH:(half + 1) * DH],
                            in1=h2[:, (1 - half) * DH:(2 - half) * DH])
                        mt = transpose300(m[:], "mt")
                        hp = pp.tile([128, DH], F32, tag="mm")
                        for ci, (k0, kw) in enumerate(K_CHUNKS):
                            nc.tensor.matmul(
                                hp[:], lhsT=mt[0:kw, 128 * ci:128 * (ci + 1)],
                                rhs=wh_sb[ci][:], start=(ci == 0),
                                stop=(ci == len(K_CHUNKS) - 1))
                        tadd = sb.tile([128, DH], F32, tag="tadd")
                        nc.vector.tensor_add(
                            out=tadd[:], in0=hp[:],
                            in1=h02[:, half * DH:(half + 1) * DH])
                        nc.scalar.activation(
                            out=hn[:, half * DH:(half + 1) * DH],
                            in_=tadd[:], func=RELU)
                    nc.scalar.dma_start(
                        out=h_nxt[128 * t:128 * (t + 1), :], in_=hn[:])

            # ---------- readout H_v ----------
            def readout():
                for w in range(NWIN):
                    mv = sb.tile([128, DH], BF, tag="mvr")
                    nc.sync.dma_start(out=mv[:],
                                      in_=xm_d[128 * w:128 * (w + 1), :])
                    mt = transpose300(mv[:], "mtr")
                    vo_sb = sb.tile([DV, 128], BF, tag="vo")
                    nc.sync.dma_start(
                        out=vo_sb[:],
                        in_=vown_t.ap()[:, 128 * w:128 * (w + 1)])
                    hp = pp.tile([128, DH], F32, tag="mm")
                    nc.tensor.matmul(
                        hp[:], lhsT=vo_sb[:],
                        rhs=wo_sb[0][:], start=True, stop=False)
                    for ci in range(3):
                        k0, kw = K_CHUNKS[ci]
                        nc.tensor.matmul(
                            hp[:], lhsT=mt[0:kw, 128 * ci:128 * (ci + 1)],
                            rhs=wo_sb[ci + 1][:], start=False, stop=(ci == 2))
                    tadd = sb.tile([128, DH], F32, tag="taddr")
                    nc.vector.tensor_add(out=tadd[:], in0=hp[:], in1=bo_bc[:])
                    hv = sb.tile([128, DH], BF, tag="hvw")
                    nc.scalar.activation(out=hv[:], in_=tadd[:], func=RELU)
                    nc.scalar.dma_start(out=hv_d[128 * w:128 * (w + 1), :],
                                        in_=hv[:])

            # ---------- molecule partial sums ----------
            def molsum():
                for w in range(NMW):
                    g = gather_multi(hv_d[:],
                                     moli_sb[:, w * T_MOL:(w + 1) * T_MOL],
                                     T_MOL, "molg")
                    ps = pp.tile([128, DH], F32, tag="mm")
                    for j in range(T_MOL):
                        oh = sb.tile([128, 128], BF, tag="oh")
                        nc.vector.tensor_tensor(
                            out=oh[:],
                            in0=molo_sb[:, w * T_MOL + j:w * T_MOL + j + 1]
                            .to_broadcast([128, 128]),
                            in1=iota_f[:], op=EQ)
                        nc.tensor.matmul(ps[:], lhsT=oh[:],
                                         rhs=g[:, j * DH:(j + 1) * DH],
                                         start=(j == 0), stop=(j == T_MOL - 1))
                    zp = sb.tile([128, DH], F32, tag="zpw")
                    nc.vector.tensor_copy(out=zp[:], in_=ps[:])
                    nc.scalar.dma_start(out=zp_d[128 * w:128 * (w + 1), :],
                                        in_=zp[:])

            # ---------- final: mean + MLP ----------
            def final():
                nc.gpsimd.collective_compute(
                    "ReduceScatter", mybir.AluOpType.add, replica_groups=RG,
                    ins=[zp_d.opt()], outs=[zr_d.opt()])
                for j in range(NZT):
                    z = sb.tile([128, DH], F32, tag="z")
                    nc.sync.dma_start(out=z[:],
                                      in_=zr_d[128 * j:128 * (j + 1), :])
                    zs = sb.tile([128, DH], BF, tag="zs")
                    nc.vector.tensor_scalar_mul(zs[:], z[:],
                                                inv_sb[:, j:j + 1])
                    zt = transpose300(zs[:], "zt")
                    p1 = pp.tile([128, EMB], F32, tag="mm")
                    for ci, (k0, kw) in enumerate(K1_CHUNKS):
                        nc.tensor.matmul(
                            p1[:], lhsT=zt[0:kw, 128 * ci:128 * (ci + 1)],
                            rhs=w1_sb[ci][:], start=(ci == 0), stop=(ci == 2))
                    t1 = sb.tile([128, EMB], F32, tag="t1")
                    nc.vector.tensor_add(out=t1[:], in0=p1[:], in1=b1_bc[:])
                    z1 = sb.tile([128, EMB], BF, tag="z1")
                    nc.scalar.activation(out=z1[:], in_=t1[:], func=RELU)
                    z1t = sb.tile([128, 2 * 128], BF, tag="z1t")
                    for ci in range(2):
                        tp = pp.tile([128, 128], BF, tag="tp", bufs=3)
                        nc.tensor.transpose(
                            out=tp[:], in_=z1[:, 128 * ci:128 * (ci + 1)],
                            identity=ident[:])
                        nc.vector.tensor_copy(
                            out=z1t[:, 128 * ci:128 * (ci + 1)], in_=tp[:])
                    p2 = pp.tile([128, EMB], F32, tag="mm")
                    for ci, (k0, kw) in enumerate(K2_CHUNKS):
                        nc.tensor.matmul(
                            p2[:], lhsT=z1t[:, 128 * ci:128 * (ci + 1)],
                            rhs=w2_sb[ci][:], start=(ci == 0), stop=(ci == 1))
                    zo = sb.tile([128, EMB], F32, tag="zo")
                    nc.vector.tensor_add(out=zo[:], in0=p2[:], in1=b2_bc[:])
                    nc.sync.dma_start(out=out_d.ap()[128 * j:128 * (j + 1), :],
                                      in_=zo[:])

            # ---------- schedule ----------
            phase0()
            h_cur, h_nxt = h_ping, h_pong
            for it in range(DEPTH - 1):
                exchange_h(h_cur)
                segsum(h_cur)
                exchange_m()
                compute(h_cur, h_nxt)
                h_cur, h_nxt = h_nxt, h_cur
            exchange_h(h_cur)
            segsum(h_cur)
            readout()
            molsum()
            final()

    nc.compile()
    return nc


def make_in_maps(meta, core, inputs, ncores=8):
    bfa = lambda x: np.asarray(x, np.float32).astype(ml_dtypes.bfloat16)
    f32 = lambda x: np.ascontiguousarray(np.asarray(x, np.float32))
    n_pair = meta["n_pair_tiles"]
    in_maps = []
    for c in range(ncores):
        cd = core[c]
        idxab = cd["idxAB"].transpose(1, 0, 2).reshape(128, 2 * n_pair)
        in_maps.append({
            "VE_T": bfa(cd["VE_T"]),
            "VownT": bfa(cd["VownT"]),
            "seg_idx": np.ascontiguousarray(cd["seg_idx"].T.astype(np.int32)),
            "seg_off": np.ascontiguousarray(cd["seg_off"].T.astype(np.float32)),
            "idxAB": np.ascontiguousarray(idxab.astype(np.int32)),
            "hs_idx": np.ascontiguousarray(cd["hs_idx"].T.astype(np.int32)),
            "ms_idx": np.ascontiguousarray(cd["ms_idx"].T.astype(np.int32)),
            "mol_idx": np.ascontiguousarray(cd["mol_idx"].T.astype(np.int32)),
            "mol_off": np.ascontiguousarray(cd["mol_off"].T.astype(np.float32)),
            "inv_cnt": f32(cd["inv_cnt"]),
            "Wi": bfa(inputs["W_i"]),
            "Wh": bfa(inputs["W_h"]),
            "Wo": bfa(inputs["W_o"]),
            "W1": bfa(inputs["W1"]),
            "W2": bfa(inputs["W2"]),
            "bo": f32(inputs["b_o"]).reshape(1, -1),
            "b1": f32(inputs["b1"]).reshape(1, -1),
            "b2": f32(inputs["b2"]).reshape(1, -1),
        })
    return in_maps


# =====================================================================
# kernel() entry point
# =====================================================================
_CACHE = {}


def _meta_key(meta):
    return tuple(sorted((k, v) for k, v in meta.items()))


def kernel(**inputs):
    import numpy as _np
    inputs = {k: _np.asarray(v) if hasattr(v, "shape") else v
              for k, v in inputs.items()}
    meta, core = prep(inputs)
    key = _meta_key(meta)
    if key not in _CACHE:
        _CACHE[key] = build_kernel(meta)
    nc = _CACHE[key]
    in_maps = make_in_maps(meta, core, inputs)
    from concourse import bass_utils
    res = bass_utils.run_bass_kernel_spmd(
        nc, in_maps, core_ids=list(range(NCORES)))
    out = _np.concatenate(
        [_np.asarray(res.results[c]["out"]) for c in range(NCORES)], axis=0)
    return out.astype(_np.float32)

